# revision 7
# baseline (speedup 1.0000x reference)
"""Trainium2 Bass kernel for nn_CombinedCriterionAEImpulse (retrieval_knn).

On-device work (8 cores, rows of pred sharded):
  q[i, j]     = 2*p_i . g_j - |g_j|^2  over (8192, 32768) pred x gt   (NxL)
  qself[i, j] = 2*p_i . p_j - |p_j|^2  over (8192, 8192)  pred x pred (NxN)
computed as K=11 bf16 hi/lo matmuls (fp32-accurate q) with the PE in 4-way
row-tiled mode (tile_position): each 2048-col PSUM supertile's four 512-col
matmuls run concurrently on tiles (0,0),(32,0),(64,0),(96,0), fed from the
four SBUF partition quadrants.

PSUM evacuation (the bottleneck: only DVE + ACT can read PSUM, ~1 f32/lane/
cycle each) is split per row-block r:
  - NxN supertiles (4) + the first NxL supertile: DVE grouped tensor_reduce
    (max over groups of 64) straight from PSUM, fp32 -> exact group maxima.
  - remaining 15 NxL supertiles: ACT copies PSUM -> SBUF bf16; DVE merges the
    copies with a tensor_tensor max chain (bf16 runs in 2x perf mode), then
    one grouped reduce. The merged slots cover 15*64=960 gt columns each.
The host resolves argmax slots (top-2) with exact recomputation, so bf16 on
the NxL path only influences candidate selection, not the final arithmetic.
Repulsion (NxN) group maxima stay fp32 end-to-end.
"""

import numpy as np

try:
    import concourse.bass as bass  # noqa: F401
except ImportError:  # pragma: no cover
    import sys

    sys.path.insert(0, "/opt/trn_rl_repo")
    import concourse.bass as bass  # noqa: F401

import concourse.mybir as mybir
import concourse.tile as tile
from concourse import bacc
from concourse.bass_utils import run_bass_kernel_spmd

P = 128
F32 = mybir.dt.float32
BF16 = mybir.dt.bfloat16
K = 11

NPRED = 8192
NGT = 32768
NCORES = 8
RPC = NPRED // NCORES  # rows per core = 1024
BLOCKS = RPC // P  # 8 row-blocks of 128
G = 64  # columns per group
ST = 2048  # supertile columns (4 PSUM banks)
SLOTS = ST // G  # 32 group slots per supertile
NXL_ST = NGT // ST  # 16
NXN_ST = NPRED // ST  # 4
N_V_NXL = 1  # leading NxL supertiles per row-block on the DVE-direct lane

GL_SLOTS = NXL_ST * SLOTS  # 512 slot columns per row
GN_SLOTS = NXN_ST * SLOTS  # 128

ALPHA = 100.0
MARGIN = 0.3
EPS = 1e-05
NEG = -3.0e38

# per-row-block supertile order: chain (ACT-copy) supertiles at both edges so
# row-block boundaries keep the copy stream flowing; DVE-direct supertiles
# (NxN + the V-lane NxL) spread mid-stream.
_ORDER = [
    ("L", 1), ("L", 2), ("N", 0), ("L", 3), ("L", 4), ("L", 0),
    ("L", 5), ("L", 6), ("N", 1), ("L", 7), ("L", 8), ("N", 2),
    ("L", 9), ("L", 10), ("N", 3), ("L", 11), ("L", 12), ("L", 13),
    ("L", 14), ("L", 15),
]

# set by test harness to capture a profile
TRACE = False
LAST_RESULTS = None


def _build_kernel():
    nc = bacc.Bacc("TRN2", debug=False, enable_asserts=False)

    xt = nc.dram_tensor("xt", [P, RPC], BF16, kind="ExternalInput").ap()
    yt = nc.dram_tensor("yt", [P, NGT // 4], BF16, kind="ExternalInput").ap()
    pt = nc.dram_tensor("pt", [P, NPRED // 4], BF16, kind="ExternalInput").ap()
    gl = nc.dram_tensor("gl", [P, BLOCKS * GL_SLOTS], BF16, kind="ExternalOutput").ap()
    gn = nc.dram_tensor("gn", [P, BLOCKS * GN_SLOTS], F32, kind="ExternalOutput").ap()

    with tile.TileContext(nc) as tc:
        with (
            tc.tile_pool(name="consts", bufs=1) as consts,
            tc.tile_pool(name="psum", bufs=2, space="PSUM") as psum,
            tc.tile_pool(name="cpp", bufs=6) as cpp,
            tc.tile_pool(name="mrgp", bufs=3) as mrgp,
            tc.tile_pool(name="acc", bufs=1) as accp,
        ):
            xt_s = consts.tile([P, RPC], BF16, tag="xt")
            nc.sync.dma_start(xt_s[:], xt)
            yt_s = consts.tile([P, NGT // 4], BF16, tag="yt")
            # first supertile's columns land first so compute starts early
            cuts = [0, 512, 1024, 2048, 4096, 6144, 8192]
            for c0, c1 in zip(cuts, cuts[1:]):
                nc.sync.dma_start(yt_s[:, c0:c1], yt[:, c0:c1])
            pt_s = consts.tile([P, NPRED // 4], BF16, tag="pt")
            nc.sync.dma_start(pt_s[:], pt)

            glall = accp.tile([P, BLOCKS * GL_SLOTS], BF16, tag="glall")
            nc.gpsimd.memset(glall[:], NEG)
            gnall = accp.tile([P, BLOCKS * GN_SLOTS], F32, tag="gnall")
            # pre-warm the ACT function table so the one-time ACT_TABLE_LOAD
            # overlaps the input DMAs instead of stalling the first real copy
            warm = accp.tile([P, 8], F32, tag="warm")
            nc.gpsimd.memset(warm[:], 0.0)
            nc.scalar.copy(out=warm[:, 4:8], in_=warm[:, 0:4])

            def emit_mms(r, src, s):
                ps = psum.tile([P, ST], F32, tag="ps")
                for m in range(4):
                    nc.tensor.matmul(
                        out=ps[:, m * 512 : (m + 1) * 512],
                        lhsT=xt_s[32 * m : 32 * m + K, r * P : (r + 1) * P],
                        rhs=src[32 * m : 32 * m + K, s * 512 : (s + 1) * 512],
                        start=True,
                        stop=True,
                        tile_position=(32 * m, 0),
                    )
                return ps

            def grouped(ap, k=G):
                return ap.rearrange("p (g k) -> p g k", k=k)

            for r in range(BLOCKS):
                mrg = None
                pend = None
                for ph, s in _ORDER:
                    if ph == "N":
                        ps = emit_mms(r, pt_s[:], s)
                        nc.vector.tensor_reduce(
                            out=gnall[:, r * GN_SLOTS + s * SLOTS :][:, :SLOTS],
                            in_=grouped(ps[:]),
                            axis=mybir.AxisListType.X,
                            op=mybir.AluOpType.max,
                        )
                        continue
                    ps = emit_mms(r, yt_s[:], s)
                    if s < N_V_NXL:
                        nc.vector.tensor_reduce(
                            out=glall[:, r * GL_SLOTS + s * SLOTS :][:, :SLOTS],
                            in_=grouped(ps[:]),
                            axis=mybir.AxisListType.X,
                            op=mybir.AluOpType.max,
                        )
                        continue
                    cp = cpp.tile([P, ST], BF16, tag="cp")
                    nc.scalar.copy(out=cp[:], in_=ps[:])
                    if pend is None and mrg is None:
                        pend = cp
                    elif mrg is None:
                        mrg = mrgp.tile([P, ST], BF16, tag="mrg")
                        nc.vector.tensor_tensor(
                            out=mrg[:], in0=pend[:], in1=cp[:],
                            op=mybir.AluOpType.max,
                        )
                        pend = None
                    else:
                        nxt = mrgp.tile([P, ST], BF16, tag="mrg")
                        nc.vector.tensor_tensor(
                            out=nxt[:], in0=mrg[:], in1=cp[:],
                            op=mybir.AluOpType.max,
                        )
                        mrg = nxt
                # chain slots land in supertile N_V_NXL's slot range
                nc.vector.tensor_reduce(
                    out=glall[:, r * GL_SLOTS + N_V_NXL * SLOTS :][:, :SLOTS],
                    in_=grouped(mrg[:]),
                    axis=mybir.AxisListType.X,
                    op=mybir.AluOpType.max,
                )
                nc.sync.dma_start(
                    out=gl[:, r * GL_SLOTS : (r + 1) * GL_SLOTS],
                    in_=glall[:, r * GL_SLOTS : (r + 1) * GL_SLOTS],
                )
                nc.sync.dma_start(
                    out=gn[:, r * GN_SLOTS : (r + 1) * GN_SLOTS],
                    in_=gnall[:, r * GN_SLOTS : (r + 1) * GN_SLOTS],
                )
    nc.compile()
    return nc


_NC_CACHE = None


def _get_nc():
    global _NC_CACHE
    if _NC_CACHE is None:
        _NC_CACHE = _build_kernel()
    return _NC_CACHE


def _quad(x, dtype):
    """[K, C] moving-operand rows -> [128, C//4] with quadrant m holding
    columns {s*2048 + m*512 + t} at partitions 32m..32m+K-1."""
    Kr, C = x.shape
    n_st = C // ST
    v = x.reshape(Kr, n_st, 4, 512)
    out = np.zeros((P, C // 4), dtype)
    for m in range(4):
        out[32 * m : 32 * m + Kr] = v[:, :, m, :].reshape(Kr, n_st * 512)
    return out


def kernel(pred_feat, pred_decoder, input_data, gt_data):
    global LAST_RESULTS
    pred_feat = np.asarray(pred_feat, dtype=np.float32)
    gt_data = np.asarray(gt_data, dtype=np.float32)
    pred = np.ascontiguousarray(pred_feat[:, :3])
    pred_n = np.ascontiguousarray(pred_feat[:, 3:])
    gt_pts = np.ascontiguousarray(gt_data[:, :3])
    gt_nrm = np.ascontiguousarray(gt_data[:, 3:])

    import ml_dtypes

    bf = ml_dtypes.bfloat16

    def split_hi_lo(x):
        hi = x.astype(bf).astype(np.float32)
        lo = (x - hi).astype(bf).astype(np.float32)
        return hi, lo

    def rhs_rows(pts):
        """[K, n] moving-operand rows for target points pts (n, 3)."""
        hi, lo = split_hi_lo(pts)
        s = (pts.astype(np.float64) ** 2).sum(1).astype(np.float32)
        shi, slo = split_hi_lo(s)
        out = np.concatenate([hi.T, lo.T, hi.T, shi[None], slo[None]], 0)
        return out.astype(bf)

    def lhs_rows(pts):
        """[K, n] stationary rows for query points pts (n, 3)."""
        hi, lo = split_hi_lo(pts)
        ones = np.ones((1, pts.shape[0]), np.float32)
        out = np.concatenate([2 * hi.T, 2 * hi.T, 2 * lo.T, -ones, -ones], 0)
        return out.astype(bf)

    ytq = _quad(rhs_rows(gt_pts), bf)

    in_maps = []
    for k in range(NCORES):
        xtq = np.zeros((P, RPC), bf)
        xk = lhs_rows(pred[k * RPC : (k + 1) * RPC])
        for m in range(4):
            xtq[32 * m : 32 * m + K] = xk
        rolled = np.roll(pred, -k * RPC, axis=0)
        in_maps.append(
            {"xt": xtq, "yt": ytq, "pt": _quad(rhs_rows(rolled), bf)}
        )

    nc = _get_nc()
    res = run_bass_kernel_spmd(
        nc, in_maps, core_ids=list(range(NCORES)), trace=TRACE
    )
    LAST_RESULTS = res

    # ---- assemble per-row slot maxima ----
    GL = np.empty((NPRED, GL_SLOTS), np.float32)
    GN = np.empty((NPRED, GN_SLOTS), np.float32)
    for k in range(NCORES):
        glk = res.results[k]["gl"].astype(np.float32).reshape(P, BLOCKS, GL_SLOTS)
        GL[k * RPC : (k + 1) * RPC] = glk.transpose(1, 0, 2).reshape(RPC, GL_SLOTS)
        gnk = res.results[k]["gn"].reshape(P, BLOCKS, GN_SLOTS)
        GN[k * RPC : (k + 1) * RPC] = gnk.transpose(1, 0, 2).reshape(RPC, GN_SLOTS)

    rows = np.arange(NPRED)

    # ---- NxL: candidate columns per slot ----
    # slot g in [0, 32): supertile s < N_V_NXL, 64 cols each
    # slot 32+g: merge chain over supertiles N_V_NXL..15, 960 cols
    chain_sts = np.arange(N_V_NXL, NXL_ST)
    n_chain = len(chain_sts)  # 15
    cw = n_chain * G  # 960
    cand_v = (np.arange(N_V_NXL * SLOTS)[:, None] * G + np.arange(G)[None, :])
    cand_v = np.concatenate(
        [cand_v, np.broadcast_to(cand_v[:, :1], (N_V_NXL * SLOTS, cw - G))], axis=1
    )  # padded to cw with repeats
    cand_c = (
        chain_sts[None, :, None] * ST
        + np.arange(SLOTS)[:, None, None] * G
        + np.arange(G)[None, None, :]
    ).reshape(SLOTS, cw)
    cand_all = np.concatenate([cand_v, cand_c], axis=0)  # [64, cw] live slots

    # top-2 live slots per row, exact recompute
    live = (N_V_NXL + 1) * SLOTS  # 64
    GLl = GL[:, :live]
    top2 = np.argpartition(-GLl, 2, axis=1)[:, :2]  # (NPRED, 2)
    cand = cand_all[top2].reshape(NPRED, 2 * cw)  # (NPRED, 1920)
    diff = pred[:, None, :] - gt_pts[cand]
    d2 = np.einsum("ijk,ijk->ij", diff, diff)
    jstar = cand[rows, np.argmin(d2, axis=1)]

    closest = gt_pts[jstar]
    attraction = np.mean(((pred - closest) ** 2).astype(np.float64))

    # ---- normal alignment ----
    cn = gt_nrm[jstar]
    pn_norm = np.maximum(np.sqrt((pred_n**2).sum(1, keepdims=True)), EPS)
    cn_norm = np.maximum(np.sqrt((cn**2).sum(1, keepdims=True)), EPS)
    cos = ((pred_n / pn_norm) * (cn / cn_norm)).sum(1)
    norm_loss = np.mean((1.0 - cos).astype(np.float64))

    # ---- repulsion: min distance to other pred points (fp32 NxN maxima) ----
    x2 = (pred.astype(np.float64) ** 2).sum(1)
    local = rows % RPC
    gc = local // G  # contaminated slot (diagonal lives in NxN supertile 0)
    core = rows // RPC
    GN2 = GN.copy()
    GN2[rows, gc] = -np.inf
    m1 = x2 - GN2.max(axis=1)
    candn = (gc[:, None] * G + np.arange(G)[None, :] + core[:, None] * RPC) % NPRED
    diffn = pred[:, None, :] - pred[candn]
    d2n = np.einsum("ijk,ijk->ij", diffn, diffn)
    d2n[candn == rows[:, None]] = np.inf
    m2 = d2n.min(axis=1)
    min_d2 = np.minimum(m1, m2)
    min_dist = np.sqrt(np.maximum(min_d2, 0.0))
    pen = np.logaddexp(0.0, ALPHA * (MARGIN - min_dist))
    repulsion = np.mean(pen**2)

    loss = attraction + repulsion + 10.0 * norm_loss
    return np.float32(loss)


# revision 8
# speedup vs baseline: 1.0004x; 1.0004x over previous
"""Trainium2 Bass kernel for nn_CombinedCriterionAEImpulse (retrieval_knn).

On-device work (8 cores, rows of pred sharded):
  q[i, j]     = 2*p_i . g_j - |g_j|^2  over (8192, 32768) pred x gt   (NxL)
  qself[i, j] = 2*p_i . p_j - |p_j|^2  over (8192, 8192)  pred x pred (NxN)
computed as K=11 bf16 hi/lo matmuls (fp32-accurate q) with the PE in 4-way
row-tiled mode (tile_position): each 2048-col PSUM supertile's four 512-col
matmuls run concurrently on tiles (0,0),(32,0),(64,0),(96,0), fed from the
four SBUF partition quadrants.

PSUM evacuation (the bottleneck: only DVE + ACT can read PSUM, ~1 f32/lane/
cycle each) is split per row-block r:
  - NxN supertiles (4) + the first NxL supertile: DVE grouped tensor_reduce
    (max over groups of 64) straight from PSUM, fp32 -> exact group maxima.
  - remaining 15 NxL supertiles: ACT copies PSUM -> SBUF bf16; DVE merges the
    copies with a tensor_tensor max chain (bf16 runs in 2x perf mode), then
    one grouped reduce. The merged slots cover 15*64=960 gt columns each.
The host resolves argmax slots (top-2) with exact recomputation, so bf16 on
the NxL path only influences candidate selection, not the final arithmetic.
Repulsion (NxN) group maxima stay fp32 end-to-end.
"""

import numpy as np

try:
    import concourse.bass as bass  # noqa: F401
except ImportError:  # pragma: no cover
    import sys

    sys.path.insert(0, "/opt/trn_rl_repo")
    import concourse.bass as bass  # noqa: F401

import concourse.mybir as mybir
import concourse.tile as tile
from concourse import bacc
from concourse.bass_utils import run_bass_kernel_spmd

P = 128
F32 = mybir.dt.float32
BF16 = mybir.dt.bfloat16
K = 11

NPRED = 8192
NGT = 32768
NCORES = 8
RPC = NPRED // NCORES  # rows per core = 1024
BLOCKS = RPC // P  # 8 row-blocks of 128
G = 64  # columns per group
ST = 2048  # supertile columns (4 PSUM banks)
SLOTS = ST // G  # 32 group slots per supertile
NXL_ST = NGT // ST  # 16
NXN_ST = NPRED // ST  # 4
N_V_NXL = 1  # leading NxL supertiles per row-block on the DVE-direct lane

GL_SLOTS = (N_V_NXL + 1) * SLOTS  # live slot columns per row (V + chain)
GN_SLOTS = NXN_ST * SLOTS  # 128

ALPHA = 100.0
MARGIN = 0.3
EPS = 1e-05
NEG = -3.0e38

# per-row-block supertile order: chain (ACT-copy) supertiles at both edges so
# row-block boundaries keep the copy stream flowing; DVE-direct supertiles
# (NxN + the V-lane NxL) spread mid-stream.
_ORDER = [
    ("L", 1), ("L", 2), ("N", 0), ("L", 3), ("L", 4), ("L", 0),
    ("L", 5), ("L", 6), ("N", 1), ("L", 7), ("L", 8), ("N", 2),
    ("L", 9), ("L", 10), ("N", 3), ("L", 11), ("L", 12), ("L", 13),
    ("L", 14), ("L", 15),
]

# set by test harness to capture a profile
TRACE = False
LAST_RESULTS = None


def _build_kernel():
    nc = bacc.Bacc("TRN2", debug=False, enable_asserts=False)

    xt = nc.dram_tensor("xt", [P, RPC], BF16, kind="ExternalInput").ap()
    yt = nc.dram_tensor("yt", [P, NGT // 4], BF16, kind="ExternalInput").ap()
    pt = nc.dram_tensor("pt", [P, NPRED // 4], BF16, kind="ExternalInput").ap()
    gl = nc.dram_tensor("gl", [P, BLOCKS * GL_SLOTS], BF16, kind="ExternalOutput").ap()
    gn = nc.dram_tensor("gn", [P, BLOCKS * GN_SLOTS], F32, kind="ExternalOutput").ap()

    with tile.TileContext(nc) as tc:
        with (
            tc.tile_pool(name="consts", bufs=1) as consts,
            tc.tile_pool(name="psum", bufs=2, space="PSUM") as psum,
            tc.tile_pool(name="cpp", bufs=6) as cpp,
            tc.tile_pool(name="mrgp", bufs=3) as mrgp,
            tc.tile_pool(name="acc", bufs=1) as accp,
        ):
            xt_s = consts.tile([P, RPC], BF16, tag="xt")
            nc.sync.dma_start(xt_s[:], xt)
            yt_s = consts.tile([P, NGT // 4], BF16, tag="yt")
            # first supertile's columns land first so compute starts early
            cuts = [0, 512, 1024, 2048, 4096, 6144, 8192]
            for c0, c1 in zip(cuts, cuts[1:]):
                nc.sync.dma_start(yt_s[:, c0:c1], yt[:, c0:c1])
            pt_s = consts.tile([P, NPRED // 4], BF16, tag="pt")
            nc.sync.dma_start(pt_s[:], pt)

            glall = accp.tile([P, BLOCKS * GL_SLOTS], BF16, tag="glall")
            gnall = accp.tile([P, BLOCKS * GN_SLOTS], F32, tag="gnall")
            # pre-warm the ACT function table so the one-time ACT_TABLE_LOAD
            # overlaps the input DMAs instead of stalling the first real copy
            warm = accp.tile([P, 8], F32, tag="warm")
            nc.vector.memset(warm[:], 0.0)
            nc.scalar.copy(out=warm[:, 4:8], in_=warm[:, 0:4])

            def emit_mms(r, src, s):
                ps = psum.tile([P, ST], F32, tag="ps")
                for m in range(4):
                    nc.tensor.matmul(
                        out=ps[:, m * 512 : (m + 1) * 512],
                        lhsT=xt_s[32 * m : 32 * m + K, r * P : (r + 1) * P],
                        rhs=src[32 * m : 32 * m + K, s * 512 : (s + 1) * 512],
                        start=True,
                        stop=True,
                        tile_position=(32 * m, 0),
                    )
                return ps

            def grouped(ap, k=G):
                return ap.rearrange("p (g k) -> p g k", k=k)

            for r in range(BLOCKS):
                mrg = None
                pend = None
                for ph, s in _ORDER:
                    if ph == "N":
                        ps = emit_mms(r, pt_s[:], s)
                        nc.vector.tensor_reduce(
                            out=gnall[:, r * GN_SLOTS + s * SLOTS :][:, :SLOTS],
                            in_=grouped(ps[:]),
                            axis=mybir.AxisListType.X,
                            op=mybir.AluOpType.max,
                        )
                        continue
                    ps = emit_mms(r, yt_s[:], s)
                    if s < N_V_NXL:
                        nc.vector.tensor_reduce(
                            out=glall[:, r * GL_SLOTS + s * SLOTS :][:, :SLOTS],
                            in_=grouped(ps[:]),
                            axis=mybir.AxisListType.X,
                            op=mybir.AluOpType.max,
                        )
                        continue
                    cp = cpp.tile([P, ST], BF16, tag="cp")
                    nc.scalar.copy(out=cp[:], in_=ps[:])
                    if pend is None and mrg is None:
                        pend = cp
                    elif mrg is None:
                        mrg = mrgp.tile([P, ST], BF16, tag="mrg")
                        nc.vector.tensor_tensor(
                            out=mrg[:], in0=pend[:], in1=cp[:],
                            op=mybir.AluOpType.max,
                        )
                        pend = None
                    else:
                        nxt = mrgp.tile([P, ST], BF16, tag="mrg")
                        nc.vector.tensor_tensor(
                            out=nxt[:], in0=mrg[:], in1=cp[:],
                            op=mybir.AluOpType.max,
                        )
                        mrg = nxt
                # chain slots land in supertile N_V_NXL's slot range
                nc.vector.tensor_reduce(
                    out=glall[:, r * GL_SLOTS + N_V_NXL * SLOTS :][:, :SLOTS],
                    in_=grouped(mrg[:]),
                    axis=mybir.AxisListType.X,
                    op=mybir.AluOpType.max,
                )
                nc.sync.dma_start(
                    out=gl[:, r * GL_SLOTS : (r + 1) * GL_SLOTS],
                    in_=glall[:, r * GL_SLOTS : (r + 1) * GL_SLOTS],
                )
                nc.sync.dma_start(
                    out=gn[:, r * GN_SLOTS : (r + 1) * GN_SLOTS],
                    in_=gnall[:, r * GN_SLOTS : (r + 1) * GN_SLOTS],
                )
    nc.compile()
    return nc


_NC_CACHE = None


def _get_nc():
    global _NC_CACHE
    if _NC_CACHE is None:
        _NC_CACHE = _build_kernel()
    return _NC_CACHE


def _quad(x, dtype):
    """[K, C] moving-operand rows -> [128, C//4] with quadrant m holding
    columns {s*2048 + m*512 + t} at partitions 32m..32m+K-1."""
    Kr, C = x.shape
    n_st = C // ST
    v = x.reshape(Kr, n_st, 4, 512)
    out = np.zeros((P, C // 4), dtype)
    for m in range(4):
        out[32 * m : 32 * m + Kr] = v[:, :, m, :].reshape(Kr, n_st * 512)
    return out


def kernel(pred_feat, pred_decoder, input_data, gt_data):
    global LAST_RESULTS
    pred_feat = np.asarray(pred_feat, dtype=np.float32)
    gt_data = np.asarray(gt_data, dtype=np.float32)
    pred = np.ascontiguousarray(pred_feat[:, :3])
    pred_n = np.ascontiguousarray(pred_feat[:, 3:])
    gt_pts = np.ascontiguousarray(gt_data[:, :3])
    gt_nrm = np.ascontiguousarray(gt_data[:, 3:])

    import ml_dtypes

    bf = ml_dtypes.bfloat16

    def split_hi_lo(x):
        hi = x.astype(bf).astype(np.float32)
        lo = (x - hi).astype(bf).astype(np.float32)
        return hi, lo

    def rhs_rows(pts):
        """[K, n] moving-operand rows for target points pts (n, 3)."""
        hi, lo = split_hi_lo(pts)
        s = (pts.astype(np.float64) ** 2).sum(1).astype(np.float32)
        shi, slo = split_hi_lo(s)
        out = np.concatenate([hi.T, lo.T, hi.T, shi[None], slo[None]], 0)
        return out.astype(bf)

    def lhs_rows(pts):
        """[K, n] stationary rows for query points pts (n, 3)."""
        hi, lo = split_hi_lo(pts)
        ones = np.ones((1, pts.shape[0]), np.float32)
        out = np.concatenate([2 * hi.T, 2 * hi.T, 2 * lo.T, -ones, -ones], 0)
        return out.astype(bf)

    ytq = _quad(rhs_rows(gt_pts), bf)

    in_maps = []
    for k in range(NCORES):
        xtq = np.zeros((P, RPC), bf)
        xk = lhs_rows(pred[k * RPC : (k + 1) * RPC])
        for m in range(4):
            xtq[32 * m : 32 * m + K] = xk
        rolled = np.roll(pred, -k * RPC, axis=0)
        in_maps.append(
            {"xt": xtq, "yt": ytq, "pt": _quad(rhs_rows(rolled), bf)}
        )

    nc = _get_nc()
    res = run_bass_kernel_spmd(
        nc, in_maps, core_ids=list(range(NCORES)), trace=TRACE
    )
    LAST_RESULTS = res

    # ---- assemble per-row slot maxima ----
    GL = np.empty((NPRED, GL_SLOTS), np.float32)
    GN = np.empty((NPRED, GN_SLOTS), np.float32)
    for k in range(NCORES):
        glk = res.results[k]["gl"].astype(np.float32).reshape(P, BLOCKS, GL_SLOTS)
        GL[k * RPC : (k + 1) * RPC] = glk.transpose(1, 0, 2).reshape(RPC, GL_SLOTS)
        gnk = res.results[k]["gn"].reshape(P, BLOCKS, GN_SLOTS)
        GN[k * RPC : (k + 1) * RPC] = gnk.transpose(1, 0, 2).reshape(RPC, GN_SLOTS)

    rows = np.arange(NPRED)

    # ---- NxL: candidate columns per slot ----
    # slot g in [0, 32): supertile s < N_V_NXL, 64 cols each
    # slot 32+g: merge chain over supertiles N_V_NXL..15, 960 cols
    chain_sts = np.arange(N_V_NXL, NXL_ST)
    n_chain = len(chain_sts)  # 15
    cw = n_chain * G  # 960
    cand_v = (np.arange(N_V_NXL * SLOTS)[:, None] * G + np.arange(G)[None, :])
    cand_v = np.concatenate(
        [cand_v, np.broadcast_to(cand_v[:, :1], (N_V_NXL * SLOTS, cw - G))], axis=1
    )  # padded to cw with repeats
    cand_c = (
        chain_sts[None, :, None] * ST
        + np.arange(SLOTS)[:, None, None] * G
        + np.arange(G)[None, None, :]
    ).reshape(SLOTS, cw)
    cand_all = np.concatenate([cand_v, cand_c], axis=0)  # [64, cw] live slots

    # top-2 live slots per row, exact recompute
    live = (N_V_NXL + 1) * SLOTS  # 64
    GLl = GL[:, :live]
    top2 = np.argpartition(-GLl, 2, axis=1)[:, :2]  # (NPRED, 2)
    cand = cand_all[top2].reshape(NPRED, 2 * cw)  # (NPRED, 1920)
    diff = pred[:, None, :] - gt_pts[cand]
    d2 = np.einsum("ijk,ijk->ij", diff, diff)
    jstar = cand[rows, np.argmin(d2, axis=1)]

    closest = gt_pts[jstar]
    attraction = np.mean(((pred - closest) ** 2).astype(np.float64))

    # ---- normal alignment ----
    cn = gt_nrm[jstar]
    pn_norm = np.maximum(np.sqrt((pred_n**2).sum(1, keepdims=True)), EPS)
    cn_norm = np.maximum(np.sqrt((cn**2).sum(1, keepdims=True)), EPS)
    cos = ((pred_n / pn_norm) * (cn / cn_norm)).sum(1)
    norm_loss = np.mean((1.0 - cos).astype(np.float64))

    # ---- repulsion: min distance to other pred points (fp32 NxN maxima) ----
    x2 = (pred.astype(np.float64) ** 2).sum(1)
    local = rows % RPC
    gc = local // G  # contaminated slot (diagonal lives in NxN supertile 0)
    core = rows // RPC
    GN2 = GN.copy()
    GN2[rows, gc] = -np.inf
    m1 = x2 - GN2.max(axis=1)
    candn = (gc[:, None] * G + np.arange(G)[None, :] + core[:, None] * RPC) % NPRED
    diffn = pred[:, None, :] - pred[candn]
    d2n = np.einsum("ijk,ijk->ij", diffn, diffn)
    d2n[candn == rows[:, None]] = np.inf
    m2 = d2n.min(axis=1)
    min_d2 = np.minimum(m1, m2)
    min_dist = np.sqrt(np.maximum(min_d2, 0.0))
    pen = np.logaddexp(0.0, ALPHA * (MARGIN - min_dist))
    repulsion = np.mean(pen**2)

    loss = attraction + repulsion + 10.0 * norm_loss
    return np.float32(loss)


# revision 9
# speedup vs baseline: 1.0166x; 1.0162x over previous
"""Trainium2 Bass kernel for nn_CombinedCriterionAEImpulse (retrieval_knn).

On-device work (8 cores, rows of pred sharded):
  q[i, j]     = 2*p_i . g_j - |g_j|^2  over (8192, 32768) pred x gt   (NxL)
  qself[i, j] = 2*p_i . p_j - |p_j|^2  over (8192, 8192)  pred x pred (NxN)
computed as K=11 bf16 hi/lo matmuls (fp32-accurate q) with the PE in 4-way
row-tiled mode (tile_position): each 2048-col PSUM supertile's four 512-col
matmuls run concurrently on tiles (0,0),(32,0),(64,0),(96,0), fed from the
four SBUF partition quadrants.

PSUM evacuation (the bottleneck: only DVE + ACT can read PSUM, ~1 f32/lane/
cycle each) is split per row-block r:
  - NxN supertiles (4) + the first NxL supertile: DVE grouped tensor_reduce
    (max over groups of 64) straight from PSUM, fp32 -> exact group maxima.
  - remaining 15 NxL supertiles: ACT copies PSUM -> SBUF bf16; DVE merges the
    copies with a tensor_tensor max chain (bf16 runs in 2x perf mode), then
    one grouped reduce. The merged slots cover 15*64=960 gt columns each.
The host resolves argmax slots (top-2) with exact recomputation, so bf16 on
the NxL path only influences candidate selection, not the final arithmetic.
Repulsion (NxN) group maxima stay fp32 end-to-end.
"""

import numpy as np

try:
    import concourse.bass as bass  # noqa: F401
except ImportError:  # pragma: no cover
    import sys

    sys.path.insert(0, "/opt/trn_rl_repo")
    import concourse.bass as bass  # noqa: F401

import concourse.mybir as mybir
import concourse.tile as tile
from concourse import bacc
from concourse.bass_utils import run_bass_kernel_spmd

P = 128
F32 = mybir.dt.float32
BF16 = mybir.dt.bfloat16
K = 11

NPRED = 8192
NGT = 32768
NCORES = 8
RPC = NPRED // NCORES  # rows per core = 1024
BLOCKS = RPC // P  # 8 row-blocks of 128
G = 64  # columns per group
ST = 2048  # supertile columns (4 PSUM banks)
SLOTS = ST // G  # 32 group slots per supertile
NXL_ST = NGT // ST  # 16
NXN_ST = NPRED // ST  # 4
N_V_NXL = 1  # leading NxL supertiles per row-block on the DVE-direct lane

GL_SLOTS = (N_V_NXL + 1) * SLOTS  # live slot columns per row (V + chain)
GN_SLOTS = NXN_ST * SLOTS  # 128

ALPHA = 100.0
MARGIN = 0.3
EPS = 1e-05
NEG = -3.0e38

# per-row-block supertile order: chain (ACT-copy) supertiles at both edges so
# row-block boundaries keep the copy stream flowing; DVE-direct supertiles
# (NxN + the V-lane NxL) spread mid-stream.
_ORDER = [
    ("L", 1), ("L", 2), ("L", 3), ("N", 0),
    ("L", 4), ("L", 5), ("L", 6), ("N", 1),
    ("L", 7), ("L", 8), ("L", 9), ("L", 0),
    ("L", 10), ("L", 11), ("L", 12), ("N", 2),
    ("L", 13), ("L", 14), ("L", 15), ("N", 3),
]

# set by test harness to capture a profile
TRACE = False
LAST_RESULTS = None


def _build_kernel():
    nc = bacc.Bacc("TRN2", debug=False, enable_asserts=False)

    xt = nc.dram_tensor("xt", [P, RPC], BF16, kind="ExternalInput").ap()
    yt = nc.dram_tensor("yt", [P, NGT // 4], BF16, kind="ExternalInput").ap()
    pt = nc.dram_tensor("pt", [P, NPRED // 4], BF16, kind="ExternalInput").ap()
    gl = nc.dram_tensor("gl", [P, BLOCKS * GL_SLOTS], BF16, kind="ExternalOutput").ap()
    gn = nc.dram_tensor("gn", [P, BLOCKS * GN_SLOTS], F32, kind="ExternalOutput").ap()

    with tile.TileContext(nc) as tc:
        with (
            tc.tile_pool(name="consts", bufs=1) as consts,
            tc.tile_pool(name="psum", bufs=2, space="PSUM") as psum,
            tc.tile_pool(name="cpp", bufs=6) as cpp,
            tc.tile_pool(name="mrgp", bufs=3) as mrgp,
            tc.tile_pool(name="acc", bufs=1) as accp,
        ):
            xt_s = consts.tile([P, RPC], BF16, tag="xt")
            nc.sync.dma_start(xt_s[:], xt)
            yt_s = consts.tile([P, NGT // 4], BF16, tag="yt")
            # first supertile's columns land first so compute starts early
            cuts = [0, 512, 1024, 2048, 4096, 6144, 8192]
            for c0, c1 in zip(cuts, cuts[1:]):
                nc.sync.dma_start(yt_s[:, c0:c1], yt[:, c0:c1])
            pt_s = consts.tile([P, NPRED // 4], BF16, tag="pt")
            nc.sync.dma_start(pt_s[:], pt)

            glall = accp.tile([P, BLOCKS * GL_SLOTS], BF16, tag="glall")
            gnall = accp.tile([P, BLOCKS * GN_SLOTS], F32, tag="gnall")
            # pre-warm the ACT function table so the one-time ACT_TABLE_LOAD
            # overlaps the input DMAs instead of stalling the first real copy
            warm = accp.tile([P, 8], F32, tag="warm")
            nc.vector.memset(warm[:], 0.0)
            nc.scalar.copy(out=warm[:, 4:8], in_=warm[:, 0:4])

            def emit_mms(r, src, s):
                ps = psum.tile([P, ST], F32, tag="ps")
                for m in range(4):
                    nc.tensor.matmul(
                        out=ps[:, m * 512 : (m + 1) * 512],
                        lhsT=xt_s[32 * m : 32 * m + K, r * P : (r + 1) * P],
                        rhs=src[32 * m : 32 * m + K, s * 512 : (s + 1) * 512],
                        start=True,
                        stop=True,
                        tile_position=(32 * m, 0),
                    )
                return ps

            def grouped(ap, k=G):
                return ap.rearrange("p (g k) -> p g k", k=k)

            for r in range(BLOCKS):
                mrg = None
                pend = None
                for ph, s in _ORDER:
                    if ph == "N":
                        ps = emit_mms(r, pt_s[:], s)
                        nc.vector.tensor_reduce(
                            out=gnall[:, r * GN_SLOTS + s * SLOTS :][:, :SLOTS],
                            in_=grouped(ps[:]),
                            axis=mybir.AxisListType.X,
                            op=mybir.AluOpType.max,
                        )
                        continue
                    ps = emit_mms(r, yt_s[:], s)
                    if s < N_V_NXL:
                        nc.vector.tensor_reduce(
                            out=glall[:, r * GL_SLOTS + s * SLOTS :][:, :SLOTS],
                            in_=grouped(ps[:]),
                            axis=mybir.AxisListType.X,
                            op=mybir.AluOpType.max,
                        )
                        continue
                    cp = cpp.tile([P, ST], BF16, tag="cp")
                    nc.scalar.copy(out=cp[:], in_=ps[:])
                    if pend is None and mrg is None:
                        pend = cp
                    elif mrg is None:
                        mrg = mrgp.tile([P, ST], BF16, tag="mrg")
                        nc.vector.tensor_tensor(
                            out=mrg[:], in0=pend[:], in1=cp[:],
                            op=mybir.AluOpType.max,
                        )
                        pend = None
                    else:
                        nxt = mrgp.tile([P, ST], BF16, tag="mrg")
                        nc.vector.tensor_tensor(
                            out=nxt[:], in0=mrg[:], in1=cp[:],
                            op=mybir.AluOpType.max,
                        )
                        mrg = nxt
                # chain slots land in supertile N_V_NXL's slot range
                nc.vector.tensor_reduce(
                    out=glall[:, r * GL_SLOTS + N_V_NXL * SLOTS :][:, :SLOTS],
                    in_=grouped(mrg[:]),
                    axis=mybir.AxisListType.X,
                    op=mybir.AluOpType.max,
                )
                nc.sync.dma_start(
                    out=gl[:, r * GL_SLOTS : (r + 1) * GL_SLOTS],
                    in_=glall[:, r * GL_SLOTS : (r + 1) * GL_SLOTS],
                )
                nc.sync.dma_start(
                    out=gn[:, r * GN_SLOTS : (r + 1) * GN_SLOTS],
                    in_=gnall[:, r * GN_SLOTS : (r + 1) * GN_SLOTS],
                )
    nc.compile()
    return nc


_NC_CACHE = None


def _get_nc():
    global _NC_CACHE
    if _NC_CACHE is None:
        _NC_CACHE = _build_kernel()
    return _NC_CACHE


def _quad(x, dtype):
    """[K, C] moving-operand rows -> [128, C//4] with quadrant m holding
    columns {s*2048 + m*512 + t} at partitions 32m..32m+K-1."""
    Kr, C = x.shape
    n_st = C // ST
    v = x.reshape(Kr, n_st, 4, 512)
    out = np.zeros((P, C // 4), dtype)
    for m in range(4):
        out[32 * m : 32 * m + Kr] = v[:, :, m, :].reshape(Kr, n_st * 512)
    return out


def kernel(pred_feat, pred_decoder, input_data, gt_data):
    global LAST_RESULTS
    pred_feat = np.asarray(pred_feat, dtype=np.float32)
    gt_data = np.asarray(gt_data, dtype=np.float32)
    pred = np.ascontiguousarray(pred_feat[:, :3])
    pred_n = np.ascontiguousarray(pred_feat[:, 3:])
    gt_pts = np.ascontiguousarray(gt_data[:, :3])
    gt_nrm = np.ascontiguousarray(gt_data[:, 3:])

    import ml_dtypes

    bf = ml_dtypes.bfloat16

    def split_hi_lo(x):
        hi = x.astype(bf).astype(np.float32)
        lo = (x - hi).astype(bf).astype(np.float32)
        return hi, lo

    def rhs_rows(pts):
        """[K, n] moving-operand rows for target points pts (n, 3)."""
        hi, lo = split_hi_lo(pts)
        s = (pts.astype(np.float64) ** 2).sum(1).astype(np.float32)
        shi, slo = split_hi_lo(s)
        out = np.concatenate([hi.T, lo.T, hi.T, shi[None], slo[None]], 0)
        return out.astype(bf)

    def lhs_rows(pts):
        """[K, n] stationary rows for query points pts (n, 3)."""
        hi, lo = split_hi_lo(pts)
        ones = np.ones((1, pts.shape[0]), np.float32)
        out = np.concatenate([2 * hi.T, 2 * hi.T, 2 * lo.T, -ones, -ones], 0)
        return out.astype(bf)

    ytq = _quad(rhs_rows(gt_pts), bf)

    in_maps = []
    for k in range(NCORES):
        xtq = np.zeros((P, RPC), bf)
        xk = lhs_rows(pred[k * RPC : (k + 1) * RPC])
        for m in range(4):
            xtq[32 * m : 32 * m + K] = xk
        rolled = np.roll(pred, -k * RPC, axis=0)
        in_maps.append(
            {"xt": xtq, "yt": ytq, "pt": _quad(rhs_rows(rolled), bf)}
        )

    nc = _get_nc()
    res = run_bass_kernel_spmd(
        nc, in_maps, core_ids=list(range(NCORES)), trace=TRACE
    )
    LAST_RESULTS = res

    # ---- assemble per-row slot maxima ----
    GL = np.empty((NPRED, GL_SLOTS), np.float32)
    GN = np.empty((NPRED, GN_SLOTS), np.float32)
    for k in range(NCORES):
        glk = res.results[k]["gl"].astype(np.float32).reshape(P, BLOCKS, GL_SLOTS)
        GL[k * RPC : (k + 1) * RPC] = glk.transpose(1, 0, 2).reshape(RPC, GL_SLOTS)
        gnk = res.results[k]["gn"].reshape(P, BLOCKS, GN_SLOTS)
        GN[k * RPC : (k + 1) * RPC] = gnk.transpose(1, 0, 2).reshape(RPC, GN_SLOTS)

    rows = np.arange(NPRED)

    # ---- NxL: candidate columns per slot ----
    # slot g in [0, 32): supertile s < N_V_NXL, 64 cols each
    # slot 32+g: merge chain over supertiles N_V_NXL..15, 960 cols
    chain_sts = np.arange(N_V_NXL, NXL_ST)
    n_chain = len(chain_sts)  # 15
    cw = n_chain * G  # 960
    cand_v = (np.arange(N_V_NXL * SLOTS)[:, None] * G + np.arange(G)[None, :])
    cand_v = np.concatenate(
        [cand_v, np.broadcast_to(cand_v[:, :1], (N_V_NXL * SLOTS, cw - G))], axis=1
    )  # padded to cw with repeats
    cand_c = (
        chain_sts[None, :, None] * ST
        + np.arange(SLOTS)[:, None, None] * G
        + np.arange(G)[None, None, :]
    ).reshape(SLOTS, cw)
    cand_all = np.concatenate([cand_v, cand_c], axis=0)  # [64, cw] live slots

    # top-2 live slots per row, exact recompute
    live = (N_V_NXL + 1) * SLOTS  # 64
    GLl = GL[:, :live]
    top2 = np.argpartition(-GLl, 2, axis=1)[:, :2]  # (NPRED, 2)
    cand = cand_all[top2].reshape(NPRED, 2 * cw)  # (NPRED, 1920)
    diff = pred[:, None, :] - gt_pts[cand]
    d2 = np.einsum("ijk,ijk->ij", diff, diff)
    jstar = cand[rows, np.argmin(d2, axis=1)]

    closest = gt_pts[jstar]
    attraction = np.mean(((pred - closest) ** 2).astype(np.float64))

    # ---- normal alignment ----
    cn = gt_nrm[jstar]
    pn_norm = np.maximum(np.sqrt((pred_n**2).sum(1, keepdims=True)), EPS)
    cn_norm = np.maximum(np.sqrt((cn**2).sum(1, keepdims=True)), EPS)
    cos = ((pred_n / pn_norm) * (cn / cn_norm)).sum(1)
    norm_loss = np.mean((1.0 - cos).astype(np.float64))

    # ---- repulsion: min distance to other pred points (fp32 NxN maxima) ----
    x2 = (pred.astype(np.float64) ** 2).sum(1)
    local = rows % RPC
    gc = local // G  # contaminated slot (diagonal lives in NxN supertile 0)
    core = rows // RPC
    GN2 = GN.copy()
    GN2[rows, gc] = -np.inf
    m1 = x2 - GN2.max(axis=1)
    candn = (gc[:, None] * G + np.arange(G)[None, :] + core[:, None] * RPC) % NPRED
    diffn = pred[:, None, :] - pred[candn]
    d2n = np.einsum("ijk,ijk->ij", diffn, diffn)
    d2n[candn == rows[:, None]] = np.inf
    m2 = d2n.min(axis=1)
    min_d2 = np.minimum(m1, m2)
    min_dist = np.sqrt(np.maximum(min_d2, 0.0))
    pen = np.logaddexp(0.0, ALPHA * (MARGIN - min_dist))
    repulsion = np.mean(pen**2)

    loss = attraction + repulsion + 10.0 * norm_loss
    return np.float32(loss)


# revision 10
# speedup vs baseline: 1.0262x; 1.0094x over previous
"""Trainium2 Bass kernel for nn_CombinedCriterionAEImpulse (retrieval_knn).

On-device work (8 cores, rows of pred sharded):
  q[i, j]     = 2*p_i . g_j - |g_j|^2  over (8192, 32768) pred x gt   (NxL)
  qself[i, j] = 2*p_i . p_j - |p_j|^2  over (8192, 8192)  pred x pred (NxN)
computed as K=11 bf16 hi/lo matmuls (fp32-accurate q) with the PE in 4-way
row-tiled mode (tile_position): each 2048-col PSUM supertile's four 512-col
matmuls run concurrently on tiles (0,0),(32,0),(64,0),(96,0), fed from the
four SBUF partition quadrants.

PSUM evacuation (the bottleneck: only DVE + ACT can read PSUM, ~1 f32/lane/
cycle each) is split per row-block r:
  - NxN supertiles (4) + the first NxL supertile: DVE grouped tensor_reduce
    (max over groups of 64) straight from PSUM, fp32 -> exact group maxima.
  - remaining 15 NxL supertiles: ACT copies PSUM -> SBUF bf16; DVE merges the
    copies with a tensor_tensor max chain (bf16 runs in 2x perf mode), then
    one grouped reduce. The merged slots cover 15*64=960 gt columns each.
The host resolves argmax slots (top-2) with exact recomputation, so bf16 on
the NxL path only influences candidate selection, not the final arithmetic.
Repulsion (NxN) group maxima stay fp32 end-to-end.
"""

import numpy as np

try:
    import concourse.bass as bass  # noqa: F401
except ImportError:  # pragma: no cover
    import sys

    sys.path.insert(0, "/opt/trn_rl_repo")
    import concourse.bass as bass  # noqa: F401

import concourse.mybir as mybir
import concourse.tile as tile
from concourse import bacc
from concourse.bass_utils import run_bass_kernel_spmd

P = 128
F32 = mybir.dt.float32
BF16 = mybir.dt.bfloat16
K = 11

NPRED = 8192
NGT = 32768
NCORES = 8
RPC = NPRED // NCORES  # rows per core = 1024
BLOCKS = RPC // P  # 8 row-blocks of 128
G = 64  # columns per group
ST = 2048  # supertile columns (4 PSUM banks)
SLOTS = ST // G  # 32 group slots per supertile
NXL_ST = NGT // ST  # 16
NXN_ST = NPRED // ST  # 4
N_V_NXL = 2  # leading NxL supertiles per row-block on the DVE-direct lane

GL_SLOTS = (N_V_NXL + 1) * SLOTS  # live slot columns per row (V + chain)
GN_SLOTS = NXN_ST * SLOTS  # 128

ALPHA = 100.0
MARGIN = 0.3
EPS = 1e-05
NEG = -3.0e38

# per-row-block supertile order: chain (ACT-copy) supertiles at both edges so
# row-block boundaries keep the copy stream flowing; DVE-direct supertiles
# (NxN + the V-lane NxL) spread mid-stream.
_ORDER = [
    ("L", 2), ("L", 3), ("N", 0),
    ("L", 4), ("L", 5), ("N", 1),
    ("L", 6), ("L", 7), ("L", 0),
    ("L", 8), ("L", 9), ("N", 2),
    ("L", 10), ("L", 11), ("L", 1),
    ("L", 12), ("L", 13), ("N", 3),
    ("L", 14), ("L", 15),
]

# set by test harness to capture a profile
TRACE = False
LAST_RESULTS = None


def _build_kernel():
    nc = bacc.Bacc("TRN2", debug=False, enable_asserts=False)

    xt = nc.dram_tensor("xt", [P, RPC], BF16, kind="ExternalInput").ap()
    yt = nc.dram_tensor("yt", [P, NGT // 4], BF16, kind="ExternalInput").ap()
    pt = nc.dram_tensor("pt", [P, NPRED // 4], BF16, kind="ExternalInput").ap()
    gl = nc.dram_tensor("gl", [P, BLOCKS * GL_SLOTS], BF16, kind="ExternalOutput").ap()
    gn = nc.dram_tensor("gn", [P, BLOCKS * GN_SLOTS], F32, kind="ExternalOutput").ap()

    with tile.TileContext(nc) as tc:
        with (
            tc.tile_pool(name="consts", bufs=1) as consts,
            tc.tile_pool(name="psum", bufs=2, space="PSUM") as psum,
            tc.tile_pool(name="cpp", bufs=6) as cpp,
            tc.tile_pool(name="mrgp", bufs=3) as mrgp,
            tc.tile_pool(name="acc", bufs=1) as accp,
        ):
            xt_s = consts.tile([P, RPC], BF16, tag="xt")
            nc.sync.dma_start(xt_s[:], xt)
            yt_s = consts.tile([P, NGT // 4], BF16, tag="yt")
            # first supertile's columns land first so compute starts early
            cuts = [0, 512, 1024, 2048, 4096, 6144, 8192]
            for c0, c1 in zip(cuts, cuts[1:]):
                nc.sync.dma_start(yt_s[:, c0:c1], yt[:, c0:c1])
            pt_s = consts.tile([P, NPRED // 4], BF16, tag="pt")
            nc.sync.dma_start(pt_s[:], pt)

            glall = accp.tile([P, BLOCKS * GL_SLOTS], BF16, tag="glall")
            gnall = accp.tile([P, BLOCKS * GN_SLOTS], F32, tag="gnall")
            # pre-warm the ACT function table so the one-time ACT_TABLE_LOAD
            # overlaps the input DMAs instead of stalling the first real copy
            warm = accp.tile([P, 8], F32, tag="warm")
            nc.vector.memset(warm[:], 0.0)
            nc.scalar.copy(out=warm[:, 4:8], in_=warm[:, 0:4])

            def emit_mms(r, src, s):
                ps = psum.tile([P, ST], F32, tag="ps")
                for m in range(4):
                    nc.tensor.matmul(
                        out=ps[:, m * 512 : (m + 1) * 512],
                        lhsT=xt_s[32 * m : 32 * m + K, r * P : (r + 1) * P],
                        rhs=src[32 * m : 32 * m + K, s * 512 : (s + 1) * 512],
                        start=True,
                        stop=True,
                        tile_position=(32 * m, 0),
                    )
                return ps

            def grouped(ap, k=G):
                return ap.rearrange("p (g k) -> p g k", k=k)

            for r in range(BLOCKS):
                mrg = None
                pend = None
                for ph, s in _ORDER:
                    if ph == "N":
                        ps = emit_mms(r, pt_s[:], s)
                        nc.vector.tensor_reduce(
                            out=gnall[:, r * GN_SLOTS + s * SLOTS :][:, :SLOTS],
                            in_=grouped(ps[:]),
                            axis=mybir.AxisListType.X,
                            op=mybir.AluOpType.max,
                        )
                        continue
                    ps = emit_mms(r, yt_s[:], s)
                    if s < N_V_NXL:
                        nc.vector.tensor_reduce(
                            out=glall[:, r * GL_SLOTS + s * SLOTS :][:, :SLOTS],
                            in_=grouped(ps[:]),
                            axis=mybir.AxisListType.X,
                            op=mybir.AluOpType.max,
                        )
                        continue
                    cp = cpp.tile([P, ST], BF16, tag="cp")
                    nc.scalar.copy(out=cp[:], in_=ps[:])
                    if pend is None and mrg is None:
                        pend = cp
                    elif mrg is None:
                        mrg = mrgp.tile([P, ST], BF16, tag="mrg")
                        nc.vector.tensor_tensor(
                            out=mrg[:], in0=pend[:], in1=cp[:],
                            op=mybir.AluOpType.max,
                        )
                        pend = None
                    else:
                        nxt = mrgp.tile([P, ST], BF16, tag="mrg")
                        nc.vector.tensor_tensor(
                            out=nxt[:], in0=mrg[:], in1=cp[:],
                            op=mybir.AluOpType.max,
                        )
                        mrg = nxt
                # chain slots land in supertile N_V_NXL's slot range
                nc.vector.tensor_reduce(
                    out=glall[:, r * GL_SLOTS + N_V_NXL * SLOTS :][:, :SLOTS],
                    in_=grouped(mrg[:]),
                    axis=mybir.AxisListType.X,
                    op=mybir.AluOpType.max,
                )
                nc.sync.dma_start(
                    out=gl[:, r * GL_SLOTS : (r + 1) * GL_SLOTS],
                    in_=glall[:, r * GL_SLOTS : (r + 1) * GL_SLOTS],
                )
                nc.sync.dma_start(
                    out=gn[:, r * GN_SLOTS : (r + 1) * GN_SLOTS],
                    in_=gnall[:, r * GN_SLOTS : (r + 1) * GN_SLOTS],
                )
    nc.compile()
    return nc


_NC_CACHE = None


def _get_nc():
    global _NC_CACHE
    if _NC_CACHE is None:
        _NC_CACHE = _build_kernel()
    return _NC_CACHE


def _quad(x, dtype):
    """[K, C] moving-operand rows -> [128, C//4] with quadrant m holding
    columns {s*2048 + m*512 + t} at partitions 32m..32m+K-1."""
    Kr, C = x.shape
    n_st = C // ST
    v = x.reshape(Kr, n_st, 4, 512)
    out = np.zeros((P, C // 4), dtype)
    for m in range(4):
        out[32 * m : 32 * m + Kr] = v[:, :, m, :].reshape(Kr, n_st * 512)
    return out


def kernel(pred_feat, pred_decoder, input_data, gt_data):
    global LAST_RESULTS
    pred_feat = np.asarray(pred_feat, dtype=np.float32)
    gt_data = np.asarray(gt_data, dtype=np.float32)
    pred = np.ascontiguousarray(pred_feat[:, :3])
    pred_n = np.ascontiguousarray(pred_feat[:, 3:])
    gt_pts = np.ascontiguousarray(gt_data[:, :3])
    gt_nrm = np.ascontiguousarray(gt_data[:, 3:])

    import ml_dtypes

    bf = ml_dtypes.bfloat16

    def split_hi_lo(x):
        hi = x.astype(bf).astype(np.float32)
        lo = (x - hi).astype(bf).astype(np.float32)
        return hi, lo

    def rhs_rows(pts):
        """[K, n] moving-operand rows for target points pts (n, 3)."""
        hi, lo = split_hi_lo(pts)
        s = (pts.astype(np.float64) ** 2).sum(1).astype(np.float32)
        shi, slo = split_hi_lo(s)
        out = np.concatenate([hi.T, lo.T, hi.T, shi[None], slo[None]], 0)
        return out.astype(bf)

    def lhs_rows(pts):
        """[K, n] stationary rows for query points pts (n, 3)."""
        hi, lo = split_hi_lo(pts)
        ones = np.ones((1, pts.shape[0]), np.float32)
        out = np.concatenate([2 * hi.T, 2 * hi.T, 2 * lo.T, -ones, -ones], 0)
        return out.astype(bf)

    ytq = _quad(rhs_rows(gt_pts), bf)

    in_maps = []
    for k in range(NCORES):
        xtq = np.zeros((P, RPC), bf)
        xk = lhs_rows(pred[k * RPC : (k + 1) * RPC])
        for m in range(4):
            xtq[32 * m : 32 * m + K] = xk
        rolled = np.roll(pred, -k * RPC, axis=0)
        in_maps.append(
            {"xt": xtq, "yt": ytq, "pt": _quad(rhs_rows(rolled), bf)}
        )

    nc = _get_nc()
    res = run_bass_kernel_spmd(
        nc, in_maps, core_ids=list(range(NCORES)), trace=TRACE
    )
    LAST_RESULTS = res

    # ---- assemble per-row slot maxima ----
    GL = np.empty((NPRED, GL_SLOTS), np.float32)
    GN = np.empty((NPRED, GN_SLOTS), np.float32)
    for k in range(NCORES):
        glk = res.results[k]["gl"].astype(np.float32).reshape(P, BLOCKS, GL_SLOTS)
        GL[k * RPC : (k + 1) * RPC] = glk.transpose(1, 0, 2).reshape(RPC, GL_SLOTS)
        gnk = res.results[k]["gn"].reshape(P, BLOCKS, GN_SLOTS)
        GN[k * RPC : (k + 1) * RPC] = gnk.transpose(1, 0, 2).reshape(RPC, GN_SLOTS)

    rows = np.arange(NPRED)

    # ---- NxL: candidate columns per slot ----
    # slot g in [0, 32): supertile s < N_V_NXL, 64 cols each
    # slot 32+g: merge chain over supertiles N_V_NXL..15, 960 cols
    chain_sts = np.arange(N_V_NXL, NXL_ST)
    n_chain = len(chain_sts)  # 15
    cw = n_chain * G  # 960
    cand_v = (np.arange(N_V_NXL * SLOTS)[:, None] * G + np.arange(G)[None, :])
    cand_v = np.concatenate(
        [cand_v, np.broadcast_to(cand_v[:, :1], (N_V_NXL * SLOTS, cw - G))], axis=1
    )  # padded to cw with repeats
    cand_c = (
        chain_sts[None, :, None] * ST
        + np.arange(SLOTS)[:, None, None] * G
        + np.arange(G)[None, None, :]
    ).reshape(SLOTS, cw)
    cand_all = np.concatenate([cand_v, cand_c], axis=0)  # [64, cw] live slots

    # top-2 live slots per row, exact recompute
    live = (N_V_NXL + 1) * SLOTS  # 64
    GLl = GL[:, :live]
    top2 = np.argpartition(-GLl, 2, axis=1)[:, :2]  # (NPRED, 2)
    cand = cand_all[top2].reshape(NPRED, 2 * cw)  # (NPRED, 1920)
    diff = pred[:, None, :] - gt_pts[cand]
    d2 = np.einsum("ijk,ijk->ij", diff, diff)
    jstar = cand[rows, np.argmin(d2, axis=1)]

    closest = gt_pts[jstar]
    attraction = np.mean(((pred - closest) ** 2).astype(np.float64))

    # ---- normal alignment ----
    cn = gt_nrm[jstar]
    pn_norm = np.maximum(np.sqrt((pred_n**2).sum(1, keepdims=True)), EPS)
    cn_norm = np.maximum(np.sqrt((cn**2).sum(1, keepdims=True)), EPS)
    cos = ((pred_n / pn_norm) * (cn / cn_norm)).sum(1)
    norm_loss = np.mean((1.0 - cos).astype(np.float64))

    # ---- repulsion: min distance to other pred points (fp32 NxN maxima) ----
    x2 = (pred.astype(np.float64) ** 2).sum(1)
    local = rows % RPC
    gc = local // G  # contaminated slot (diagonal lives in NxN supertile 0)
    core = rows // RPC
    GN2 = GN.copy()
    GN2[rows, gc] = -np.inf
    m1 = x2 - GN2.max(axis=1)
    candn = (gc[:, None] * G + np.arange(G)[None, :] + core[:, None] * RPC) % NPRED
    diffn = pred[:, None, :] - pred[candn]
    d2n = np.einsum("ijk,ijk->ij", diffn, diffn)
    d2n[candn == rows[:, None]] = np.inf
    m2 = d2n.min(axis=1)
    min_d2 = np.minimum(m1, m2)
    min_dist = np.sqrt(np.maximum(min_d2, 0.0))
    pen = np.logaddexp(0.0, ALPHA * (MARGIN - min_dist))
    repulsion = np.mean(pen**2)

    loss = attraction + repulsion + 10.0 * norm_loss
    return np.float32(loss)
